# revision 10
# baseline (speedup 1.0000x reference)
"""Trainium2 Bass kernel for nn_Attn_45423574123081 (sparse_attention).

Computes, for inputs enc [B=32, L=1024, D=64], W [64, 64], b [64]:
    energy = enc @ W.T + b                       # [B, L, D]
    scores[t, b, j] = energy[b, j] . enc[b, t]   # [T=1024, B, L]
    scores[t, :, t] = 0
    out = softmax(scores, axis=-1)

Sharding: data-parallel over batch; 4 batches per core on 8 NeuronCores.

v2 redesign (from 70.1us baseline trace analysis):
 * Interleaved t-layout (t = 8p + n): the enc SWDGE cast-load becomes 128
   contiguous 2KB descriptors instead of 1024x256B (the old batch-0 load
   took ~10.5us and gated the first chunk at 17.3us).
 * Paired PE transposes [128,128]: even chunks' E^T on partitions 0-63,
   odd on 64-127.  K=64 matmuls then alternate tile_position row groups
   (0,0)/(64,0) between consecutive chunks, so neighbouring score
   matmuls run CONCURRENTLY in the PE array (~2x PE throughput; the
   old kernel's PE was 78% busy and stalled ScalarE at batch seams).
 * G^T built by one pair of concurrent matmuls with stationary
   [W^T | W^T] (output duplicated to both partition halves), written to
   two PSUM banks and interleave-merged f32->f16 by two strided DVE
   copies so gt columns are in true t'-order (output DMA stays 2KB-
   contiguous per partition).
 * eb = -(E.b) per chunk-pair via one K=128 matmul against a
   block-diagonal [[-b,0],[0,-b]] rhs.
 * Diagonal (softmax shift-invariance absorbs the +E.b bias): DVE
   predicated write of -c_t onto the stride-8 diagonal view ps[:, i::8].
 * Softmax unchanged: ScalarE exp f32 PSUM -> bf16 SBUF with fused
   accumulator row sums ('A') or DVE tensor_tensor_reduce ('T', to
   offload the ~290ns READ_ACCUMULATOR when ScalarE is the pacer),
   DVE reciprocal + normalize, 512KB bf16 DMA per pair of t-blocks.
 * Output rows t = 8p + i are stride-8 in DRAM but stay 2KB-contiguous
   per partition (partition stride 64KB).
"""

import numpy as np

_B, _L, _D, _T = 32, 1024, 64, 1024
_N_CORES = 8
_BPC = _B // _N_CORES  # batches per core

# Per-chunk row-sum mode: 'A' = ScalarE accumulator (+~290ns READ_ACC on
# ScalarE), 'T' = DVE tensor_tensor_reduce (~0.6us on DVE).
_MODES = ["A", "A", "A", "A", "A", "A", "A", "A"]

_compiled_nc = None


def _build():
    global _compiled_nc
    if _compiled_nc is not None:
        return _compiled_nc

    import concourse.bacc as bacc
    import concourse.mybir as mybir
    from concourse import tile

    dt = mybir.dt
    AF = mybir.ActivationFunctionType
    ALU = mybir.AluOpType

    nc = bacc.Bacc(
        "TRN2",
        target_bir_lowering=False,
        debug=False,
        enable_asserts=False,
        num_devices=_N_CORES,
    )
    enc_d = nc.dram_tensor("enc", [_BPC, _L, _D], dt.float32, kind="ExternalInput")
    # host-precomputed stationaries: [[W^T, W^T], [W^T, W^T]] and
    # [[-b, 0], [0, -b]] (removes the whole on-device W/b prep chain
    # from the startup critical path)
    w4_d = nc.dram_tensor("w4x2", [128, 128], dt.float16, kind="ExternalInput")
    nb_d = nc.dram_tensor("nb22", [128, 2], dt.float16, kind="ExternalInput")
    id16_d = nc.dram_tensor("ident16", [128, 128], dt.float16, kind="ExternalInput")
    id8_d = nc.dram_tensor("ident8", [128, 128], dt.int8, kind="ExternalInput")
    out_d = nc.dram_tensor("out", [_T, _BPC, _L], dt.bfloat16, kind="ExternalOutput")

    with tile.TileContext(nc) as tc:
        with (
            tc.tile_pool(name="const", bufs=1) as cpool,
            tc.tile_pool(name="encp", bufs=2) as encpool,
            tc.tile_pool(name="etp", bufs=2) as etpool,
            tc.tile_pool(name="gtp", bufs=2) as gtpool,
            tc.tile_pool(name="ebp", bufs=2) as ebpool,
            tc.tile_pool(name="expp", bufs=6) as exppool,
            tc.tile_pool(name="outp", bufs=3) as outpool,
            tc.tile_pool(name="sump", bufs=2) as sumpool,
            tc.tile_pool(name="scrp", bufs=2) as scrpool,
            tc.tile_pool(name="ps_s", bufs=3, space="PSUM") as ps_s_pool,
            tc.tile_pool(name="ps_m", bufs=2, space="PSUM") as ps_m_pool,
        ):
            # Dummy exp at t=0 hoists the ~2.7us ACT_TABLE_LOAD off the
            # first chunk's critical path.
            warm = cpool.tile([1, 2], dt.float32)
            nc.vector.memset(warm[:], 0.0)
            nc.scalar.activation(warm[:, 0:1], warm[:, 1:2], AF.Exp)

            # enc batch 0 goes f32 over the fast HWDGE sync queue (2KB
            # contiguous per partition) and is cast f32->f16 on the
            # still-idle DVE; batches 1-3 use the SWDGE casting loads
            # off the critical path.  Interleaved layout: partition p
            # holds rows t = 8p+0..8p+7.
            enc32_b0 = cpool.tile([128, 8 * _D], dt.float32)
            nc.sync.dma_start(
                enc32_b0[:].rearrange("p (n d) -> p n d", n=8),
                enc_d[0].rearrange("(p n) d -> p n d", p=128),
            )
            ident16 = cpool.tile([128, 128], dt.float16)
            nc.sync.dma_start(ident16[:], id16_d[:])
            w4x2 = cpool.tile([128, 128], dt.float16)
            nc.sync.dma_start(w4x2[:], w4_d[:])
            nb22 = cpool.tile([128, 2], dt.float16)
            nc.sync.dma_start(nb22[:], nb_d[:])
            ident_i8 = cpool.tile([128, 128], dt.int8)
            nc.sync.dma_start(ident_i8[:], id8_d[:])

            enc16_b0 = encpool.tile([128, 8 * _D], dt.float16, tag="enc16")
            nc.vector.tensor_copy(enc16_b0[:], enc32_b0[:])

            def prep_load(bb):
                """enc f32 DRAM -> f16 SBUF (cast on SWDGE), interleaved."""
                enc16 = encpool.tile([128, 8 * _D], dt.float16, tag="enc16")
                nc.gpsimd.dma_start(
                    enc16[:].rearrange("p (n d) -> p n d", n=8),
                    enc_d[bb].rearrange("(p n) d -> p n d", p=128),
                )
                return enc16

            def prep_tr(bb, enc16, eng=None):
                """4 paired transposes: et2 [128, 512] f16, pair q block
                at cols q*128..: rows 0-63 = E^T for chunk 2q (t=8p+2q),
                rows 64-127 = chunk 2q+1.  eng picks the PSUM->SBUF copy
                engine (ScalarE for batch 0, when it is still idle)."""
                ps_et = ps_m_pool.tile([128, 512], dt.float16, tag="ps_m")
                for q in range(4):
                    nc.tensor.transpose(
                        ps_et[:, q * 128 : (q + 1) * 128],
                        enc16[:, q * 128 : (q + 1) * 128],
                        ident16[:],
                    )
                et2 = etpool.tile([128, 512], dt.float16, tag="et2")
                if eng is None:
                    nc.vector.tensor_copy(et2[:], ps_et[:])
                else:
                    eng.copy(et2[:], ps_et[:])
                return et2

            def et_sl(et2, i):
                """lhsT slice [64, 128] for chunk i (row half i%2)."""
                q, h = i // 2, i % 2
                return et2[64 * h : 64 * h + 64, q * 128 : (q + 1) * 128]

            def prep_g(bb, et2):
                """gt [128, 1024] f16 in true t'-order, duplicated on
                both partition halves.  Two concurrent row-group MMs
                (even chunks / odd chunks) write two PSUM banks in
                stacked (q,p) order; two strided DVE copies interleave
                the columns (t' = 8p + 2q + h)."""
                gt2 = gtpool.tile([128, _L], dt.float16, tag="gt2")
                gview = gt2[:].rearrange("p (pp h) -> p h pp", h=2)
                for h in range(2):
                    ps_g = ps_m_pool.tile([128, 512], dt.float32, tag="ps_m")
                    nc.tensor.matmul(
                        ps_g[:],
                        w4x2[64 * h : 64 * h + 64, :],
                        et2[64 * h : 64 * h + 64, :],
                        start=True,
                        stop=True,
                    )
                    # src re-walked (q,p)->(p,q) so copy order matches the
                    # ascending-t' destination view
                    nc.vector.tensor_copy(
                        gview[:, h : h + 1].squeeze(1),
                        ps_g[:].rearrange("p (q pp) -> p pp q", q=4),
                    )
                return gt2

            def prep_eb(bb, et2):
                """ebn [128, 8] f32: col i = -c_t for chunk i rows
                (t = 8p + i).  One K=128 MM per chunk pair."""
                ps_eb = ps_m_pool.tile([128, 8], dt.float32, tag="ps_m")
                for q in range(4):
                    nc.tensor.matmul(
                        ps_eb[:, 2 * q : 2 * q + 2],
                        et2[:, q * 128 : (q + 1) * 128],
                        nb22[:],
                        start=True,
                        stop=True,
                    )
                ebn = ebpool.tile([128, 8], dt.float32, tag="ebn")
                nc.vector.tensor_copy(ebn[:], ps_eb[:])
                return ebn

            def chunk(bb, i, et2, gt2, ebn, sums):
                """One t-block: 2 score MMs (N=512 halves; consecutive
                chunks alternate PE row groups so they overlap), diag
                write on the stride-8 view, exp, row sum."""
                mode = _MODES[i]
                h = i % 2
                ps = ps_s_pool.tile([128, _L], dt.float32, tag="ps_s")
                for s in range(2):
                    sl = slice(s * 512, (s + 1) * 512)
                    nc.tensor.matmul(
                        ps[:, sl],
                        et_sl(et2, i),
                        gt2[64 * h : 64 * h + 64, sl],
                        start=True,
                        stop=True,
                    )
                # diagonal of chunk i sits at (p, 8p + i): stride-8 view
                diag_view = ps[:].rearrange("p (pp e) -> p e pp", e=8)[
                    :, i : i + 1
                ].squeeze(1)
                nc.vector.copy_predicated(
                    diag_view,
                    ident_i8[:],
                    ebn[:, i : i + 1].to_broadcast([128, 128]),
                )
                exp_sb = exppool.tile([128, _L], dt.bfloat16, tag="exp")
                scol = sums[:, i : i + 1]
                if mode == "A":
                    nc.scalar.activation(exp_sb[:], ps[:], AF.Exp, accum_out=scol)
                else:
                    nc.scalar.activation(exp_sb[:], ps[:], AF.Exp)
                    scr = scrpool.tile([128, 512], dt.bfloat16, tag="scr")
                    nc.vector.tensor_tensor_reduce(
                        scr[:],
                        exp_sb[:, 0:512],
                        exp_sb[:, 512:1024],
                        1.0,
                        0.0,
                        ALU.add,
                        ALU.add,
                        accum_out=scol,
                    )
                return exp_sb

            out_r = out_d.rearrange("(p e) b j -> p e b j", e=8)

            def finish_pair(bb, q, exps, sums, recips):
                """Reciprocal for chunks 2q/2q+1, normalize, DMA out."""
                pr = slice(2 * q, 2 * q + 2)
                nc.vector.reciprocal(recips[:, pr], sums[:, pr])
                out16 = outpool.tile([128, 2 * _L], dt.bfloat16, tag="o16")
                for h in range(2):
                    i = 2 * q + h
                    nc.vector.tensor_scalar_mul(
                        out16[:, h * _L : (h + 1) * _L],
                        exps[i][:],
                        recips[:, i : i + 1],
                    )
                dst = out_r[:, 2 * q : 2 * q + 2, bb : bb + 1, :].squeeze(2)
                nc.sync.dma_start(dst, out16[:].rearrange("p (e j) -> p e j", e=2))

            def finish_chunk(bb, i, exp_sb, sums, recips):
                """Tail-latency variant for the last batch: ship each
                256KB t-block as soon as its sum lands, spread across
                DMA-capable queues so the drains overlap."""
                nc.vector.reciprocal(recips[:, i : i + 1], sums[:, i : i + 1])
                out16 = outpool.tile([128, _L], dt.bfloat16, tag="o16s")
                nc.vector.tensor_scalar_mul(out16[:], exp_sb[:], recips[:, i : i + 1])
                dst = out_r[:, i : i + 1, bb : bb + 1, :].squeeze(2).squeeze(1)
                eng = {4: nc.sync, 5: nc.gpsimd, 6: nc.sync, 7: nc.scalar}[i]
                eng.dma_start(dst, out16[:])

            # --- software-pipelined emission ---------------------------------
            enc = [None] * _BPC
            et = [None] * _BPC
            gt = [None] * _BPC
            eb = [None] * _BPC
            enc[0] = enc16_b0
            enc[1] = prep_load(1)  # first op on the SWDGE queue
            et[0] = prep_tr(0, enc[0], eng=nc.scalar)
            gt[0] = prep_g(0, et[0])
            eb[0] = prep_eb(0, et[0])

            pending = []
            for bb in range(_BPC):
                sums = sumpool.tile([128, 8], dt.float32, tag="sums")
                recips = sumpool.tile([128, 8], dt.float32, tag="recips")
                exps = [None] * 8
                last = bb == _BPC - 1
                for i in range(8):
                    exps[i] = chunk(bb, i, et[bb], gt[bb], eb[bb], sums)
                    if pending:
                        # deferred pair finish: emit AFTER the next chunk's
                        # diagonal write so the in-order DVE queue never
                        # makes ScalarE wait on normalize work
                        pending.pop()()
                    if last and i >= 4:
                        finish_chunk(bb, i, exps[i], sums, recips)
                    elif i % 2 == 1:
                        pending.append(
                            lambda bb=bb, q=i // 2, e=exps, s=sums, r=recips:
                                finish_pair(bb, q, e, s, r)
                        )
                    if bb + 1 < _BPC:
                        if i == 3:
                            et[bb + 1] = prep_tr(bb + 1, enc[bb + 1])
                        elif i == 5:
                            gt[bb + 1] = prep_g(bb + 1, et[bb + 1])
                        elif i == 6:
                            eb[bb + 1] = prep_eb(bb + 1, et[bb + 1])
                    if bb + 2 < _BPC and i == 2:
                        enc[bb + 2] = prep_load(bb + 2)

    nc.compile()
    _compiled_nc = nc
    return nc


def _numpy_fallback(enc, W, b, tl):
    energy = np.einsum("bld,ed->ble", enc, W) + b
    scores = np.einsum("bjd,btd->tbj", energy, enc[:, :tl, :])
    t_idx = np.arange(tl)
    scores[t_idx, :, t_idx] = 0.0
    m = scores.max(axis=-1, keepdims=True)
    e = np.exp(scores - m)
    return (e / e.sum(axis=-1, keepdims=True)).astype(np.float32)


def _run(encoder_outputs, W, b, target_length=1024, **run_kwargs):
    enc = np.ascontiguousarray(np.asarray(encoder_outputs, dtype=np.float32))
    Wn = np.ascontiguousarray(np.asarray(W, dtype=np.float32))
    bn = np.ascontiguousarray(np.asarray(b, dtype=np.float32))
    tl = int(target_length)
    if enc.shape != (_B, _L, _D) or tl != _T:
        return _numpy_fallback(enc, Wn, bn, tl), None

    from concourse.bass_utils import run_bass_kernel_spmd

    nc = _build()
    id16 = np.eye(128, dtype=np.float16)
    id8 = np.eye(128, dtype=np.int8)
    wt16 = Wn.T.astype(np.float16)  # [d, e]
    w4x2 = np.block([[wt16, wt16], [wt16, wt16]])  # [128, 128]
    nb16 = (-bn).astype(np.float16)
    nb22 = np.zeros((128, 2), dtype=np.float16)
    nb22[0:64, 0] = nb16
    nb22[64:128, 1] = nb16
    in_maps = [
        {
            "enc": enc[i * _BPC : (i + 1) * _BPC],
            "w4x2": w4x2,
            "nb22": nb22,
            "ident16": id16,
            "ident8": id8,
        }
        for i in range(_N_CORES)
    ]
    res = run_bass_kernel_spmd(nc, in_maps, list(range(_N_CORES)), **run_kwargs)
    out = np.concatenate(
        [np.asarray(res.results[i]["out"]) for i in range(_N_CORES)], axis=1
    ).astype(np.float32)
    return out, res


def kernel(encoder_outputs, W, b, target_length=1024):
    out, _ = _run(encoder_outputs, W, b, target_length)
    return out


def kernel_profiled(encoder_outputs, W, b, target_length=1024):
    """Run with NTFF tracing; returns (output, BassKernelResults)."""
    return _run(encoder_outputs, W, b, target_length, trace=True)


# revision 14
# speedup vs baseline: 1.0061x; 1.0061x over previous
"""Trainium2 Bass kernel for nn_Attn_45423574123081 (sparse_attention).

Computes, for inputs enc [B=32, L=1024, D=64], W [64, 64], b [64]:
    energy = enc @ W.T + b                       # [B, L, D]
    scores[t, b, j] = energy[b, j] . enc[b, t]   # [T=1024, B, L]
    scores[t, :, t] = 0
    out = softmax(scores, axis=-1)

Sharding: data-parallel over batch; 4 batches per core on 8 NeuronCores.

v2 redesign (from 70.1us baseline trace analysis):
 * Interleaved t-layout (t = 8p + n): the enc SWDGE cast-load becomes 128
   contiguous 2KB descriptors instead of 1024x256B (the old batch-0 load
   took ~10.5us and gated the first chunk at 17.3us).
 * Paired PE transposes [128,128]: even chunks' E^T on partitions 0-63,
   odd on 64-127.  K=64 matmuls then alternate tile_position row groups
   (0,0)/(64,0) between consecutive chunks, so neighbouring score
   matmuls run CONCURRENTLY in the PE array (~2x PE throughput; the
   old kernel's PE was 78% busy and stalled ScalarE at batch seams).
 * G^T built by one pair of concurrent matmuls with stationary
   [W^T | W^T] (output duplicated to both partition halves), written to
   two PSUM banks and interleave-merged f32->f16 by two strided DVE
   copies so gt columns are in true t'-order (output DMA stays 2KB-
   contiguous per partition).
 * eb = -(E.b) per chunk-pair via one K=128 matmul against a
   block-diagonal [[-b,0],[0,-b]] rhs.
 * Diagonal (softmax shift-invariance absorbs the +E.b bias): DVE
   predicated write of -c_t onto the stride-8 diagonal view ps[:, i::8].
 * Softmax unchanged: ScalarE exp f32 PSUM -> bf16 SBUF with fused
   accumulator row sums ('A') or DVE tensor_tensor_reduce ('T', to
   offload the ~290ns READ_ACCUMULATOR when ScalarE is the pacer),
   DVE reciprocal + normalize, 512KB bf16 DMA per pair of t-blocks.
 * Output rows t = 8p + i are stride-8 in DRAM but stay 2KB-contiguous
   per partition (partition stride 64KB).
"""

import numpy as np

_B, _L, _D, _T = 32, 1024, 64, 1024
_N_CORES = 8
_BPC = _B // _N_CORES  # batches per core

# Per-chunk row-sum mode: 'A' = ScalarE accumulator (+~290ns READ_ACC on
# ScalarE), 'T' = DVE tensor_tensor_reduce (~0.6us on DVE).
_MODES = ["A", "A", "A", "A", "A", "A", "A", "A"]

_compiled_nc = None


def _build():
    global _compiled_nc
    if _compiled_nc is not None:
        return _compiled_nc

    import concourse.bacc as bacc
    import concourse.mybir as mybir
    from concourse import tile

    dt = mybir.dt
    AF = mybir.ActivationFunctionType
    ALU = mybir.AluOpType

    nc = bacc.Bacc(
        "TRN2",
        target_bir_lowering=False,
        debug=False,
        enable_asserts=False,
        num_devices=_N_CORES,
    )
    enc_d = nc.dram_tensor("enc", [_BPC, _L, _D], dt.float32, kind="ExternalInput")
    # host-precomputed stationaries: [[W^T, W^T], [W^T, W^T]] and
    # [[-b, 0], [0, -b]] (removes the whole on-device W/b prep chain
    # from the startup critical path)
    w4_d = nc.dram_tensor("w4x2", [128, 128], dt.float16, kind="ExternalInput")
    nb_d = nc.dram_tensor("nb22", [128, 2], dt.float16, kind="ExternalInput")
    id16_d = nc.dram_tensor("ident16", [128, 128], dt.float16, kind="ExternalInput")
    id8_d = nc.dram_tensor("ident8", [128, 128], dt.int8, kind="ExternalInput")
    out_d = nc.dram_tensor("out", [_T, _BPC, _L], dt.bfloat16, kind="ExternalOutput")

    with tile.TileContext(nc) as tc:
        with (
            tc.tile_pool(name="const", bufs=1) as cpool,
            tc.tile_pool(name="encp", bufs=2) as encpool,
            tc.tile_pool(name="etp", bufs=2) as etpool,
            tc.tile_pool(name="gtp", bufs=2) as gtpool,
            tc.tile_pool(name="ebp", bufs=2) as ebpool,
            tc.tile_pool(name="expp", bufs=6) as exppool,
            tc.tile_pool(name="outp", bufs=3) as outpool,
            tc.tile_pool(name="sump", bufs=2) as sumpool,
            tc.tile_pool(name="scrp", bufs=2) as scrpool,
            tc.tile_pool(name="ps_s", bufs=3, space="PSUM") as ps_s_pool,
            tc.tile_pool(name="ps_m", bufs=2, space="PSUM") as ps_m_pool,
        ):
            # Dummy exp at t=0 hoists the ~2.7us ACT_TABLE_LOAD off the
            # first chunk's critical path.
            warm = cpool.tile([1, 2], dt.float32)
            nc.vector.memset(warm[:], 0.0)
            nc.scalar.activation(warm[:, 0:1], warm[:, 1:2], AF.Exp)

            # enc batch 0 goes f32 over the fast HWDGE sync queue (2KB
            # contiguous per partition) and is cast f32->f16 on the
            # still-idle DVE; batches 1-3 use the SWDGE casting loads
            # off the critical path.  Interleaved layout: partition p
            # holds rows t = 8p+0..8p+7.
            ident16 = cpool.tile([128, 128], dt.float16)
            nc.sync.dma_start(ident16[:], id16_d[:])
            enc32_b0 = cpool.tile([128, 8 * _D], dt.float32)
            for h in range(2):
                nc.sync.dma_start(
                    enc32_b0[:, h * 4 * _D : (h + 1) * 4 * _D].rearrange(
                        "p (n d) -> p n d", n=4
                    ),
                    enc_d[0].rearrange("(p n) d -> p n d", p=128)[
                        :, h * 4 : (h + 1) * 4
                    ],
                )
            w4x2 = cpool.tile([128, 128], dt.float16)
            nc.sync.dma_start(w4x2[:], w4_d[:])
            nb22 = cpool.tile([128, 2], dt.float16)
            nc.sync.dma_start(nb22[:], nb_d[:])
            ident_i8 = cpool.tile([128, 128], dt.int8)
            nc.sync.dma_start(ident_i8[:], id8_d[:])

            enc16_b0 = encpool.tile([128, 8 * _D], dt.float16, tag="enc16")
            for h in range(2):
                nc.vector.tensor_copy(
                    enc16_b0[:, h * 4 * _D : (h + 1) * 4 * _D],
                    enc32_b0[:, h * 4 * _D : (h + 1) * 4 * _D],
                )

            def prep_load(bb):
                """enc f32 DRAM -> f16 SBUF (cast on SWDGE), interleaved."""
                enc16 = encpool.tile([128, 8 * _D], dt.float16, tag="enc16")
                nc.gpsimd.dma_start(
                    enc16[:].rearrange("p (n d) -> p n d", n=8),
                    enc_d[bb].rearrange("(p n) d -> p n d", p=128),
                )
                return enc16

            def prep_tr(bb, enc16, eng=None):
                """4 paired transposes: et2 [128, 512] f16, pair q block
                at cols q*128..: rows 0-63 = E^T for chunk 2q (t=8p+2q),
                rows 64-127 = chunk 2q+1.  eng picks the PSUM->SBUF copy
                engine (ScalarE for batch 0, when it is still idle)."""
                ps_et = ps_m_pool.tile([128, 512], dt.float16, tag="ps_m")
                for q in range(4):
                    nc.tensor.transpose(
                        ps_et[:, q * 128 : (q + 1) * 128],
                        enc16[:, q * 128 : (q + 1) * 128],
                        ident16[:],
                    )
                et2 = etpool.tile([128, 512], dt.float16, tag="et2")
                if eng is None:
                    nc.vector.tensor_copy(et2[:], ps_et[:])
                else:
                    eng.copy(et2[:], ps_et[:])
                return et2

            def et_sl(et2, i):
                """lhsT slice [64, 128] for chunk i (row half i%2)."""
                q, h = i // 2, i % 2
                return et2[64 * h : 64 * h + 64, q * 128 : (q + 1) * 128]

            def prep_g_mm(bb, et2):
                """G MMs: two concurrent row-group MMs (even / odd
                chunks) into two PSUM banks, stacked (q,p) col order."""
                gt2 = gtpool.tile([128, _L], dt.float16, tag="gt2")
                ps_gs = []
                for h in range(2):
                    ps_g = ps_m_pool.tile([128, 512], dt.float32, tag="ps_m")
                    nc.tensor.matmul(
                        ps_g[:],
                        w4x2[64 * h : 64 * h + 64, :],
                        et2[64 * h : 64 * h + 64, :],
                        start=True,
                        stop=True,
                    )
                    ps_gs.append(ps_g)
                return gt2, ps_gs

            def prep_g_cast(gt2, ps_gs, h):
                """Interleave-merge one parity into gt (t' = 8p+2q+h).
                Emitted one half per chunk slot so the 661ns casts never
                stack up between diag writes in the in-order DVE queue.
                src re-walked (q,p)->(p,q) to match the ascending-t'
                destination view."""
                gview = gt2[:].rearrange("p (pp h) -> p h pp", h=2)
                nc.vector.tensor_copy(
                    gview[:, h : h + 1].squeeze(1),
                    ps_gs[h][:].rearrange("p (q pp) -> p pp q", q=4),
                )

            def prep_eb(bb, et2):
                """ebn [128, 8] f32: col i = -c_t for chunk i rows
                (t = 8p + i).  One K=128 MM per chunk pair."""
                ps_eb = ps_m_pool.tile([128, 8], dt.float32, tag="ps_m")
                for q in range(4):
                    nc.tensor.matmul(
                        ps_eb[:, 2 * q : 2 * q + 2],
                        et2[:, q * 128 : (q + 1) * 128],
                        nb22[:],
                        start=True,
                        stop=True,
                    )
                ebn = ebpool.tile([128, 8], dt.float32, tag="ebn")
                nc.vector.tensor_copy(ebn[:], ps_eb[:])
                return ebn

            def chunk(bb, i, et2, gt2, ebn, sums):
                """One t-block: 2 score MMs (N=512 halves; consecutive
                chunks alternate PE row groups so they overlap), diag
                write on the stride-8 view, exp, row sum."""
                mode = _MODES[i]
                h = i % 2
                ps = ps_s_pool.tile([128, _L], dt.float32, tag="ps_s")
                for s in range(2):
                    sl = slice(s * 512, (s + 1) * 512)
                    nc.tensor.matmul(
                        ps[:, sl],
                        et_sl(et2, i),
                        gt2[64 * h : 64 * h + 64, sl],
                        start=True,
                        stop=True,
                    )
                # diagonal of chunk i sits at (p, 8p + i): stride-8 view
                diag_view = ps[:].rearrange("p (pp e) -> p e pp", e=8)[
                    :, i : i + 1
                ].squeeze(1)
                nc.vector.copy_predicated(
                    diag_view,
                    ident_i8[:],
                    ebn[:, i : i + 1].to_broadcast([128, 128]),
                )
                exp_sb = exppool.tile([128, _L], dt.bfloat16, tag="exp")
                scol = sums[:, i : i + 1]
                if mode == "A":
                    nc.scalar.activation(exp_sb[:], ps[:], AF.Exp, accum_out=scol)
                else:
                    nc.scalar.activation(exp_sb[:], ps[:], AF.Exp)
                    scr = scrpool.tile([128, 512], dt.bfloat16, tag="scr")
                    nc.vector.tensor_tensor_reduce(
                        scr[:],
                        exp_sb[:, 0:512],
                        exp_sb[:, 512:1024],
                        1.0,
                        0.0,
                        ALU.add,
                        ALU.add,
                        accum_out=scol,
                    )
                return exp_sb

            out_r = out_d.rearrange("(p e) b j -> p e b j", e=8)

            def finish_pair(bb, q, exps, sums, recips):
                """Reciprocal for chunks 2q/2q+1, normalize, DMA out."""
                pr = slice(2 * q, 2 * q + 2)
                nc.vector.reciprocal(recips[:, pr], sums[:, pr])
                out16 = outpool.tile([128, 2 * _L], dt.bfloat16, tag="o16")
                for h in range(2):
                    i = 2 * q + h
                    nc.vector.tensor_scalar_mul(
                        out16[:, h * _L : (h + 1) * _L],
                        exps[i][:],
                        recips[:, i : i + 1],
                    )
                dst = out_r[:, 2 * q : 2 * q + 2, bb : bb + 1, :].squeeze(2)
                nc.sync.dma_start(dst, out16[:].rearrange("p (e j) -> p e j", e=2))

            def finish_chunk(bb, i, exp_sb, sums, recips):
                """Tail-latency variant for the last batch: ship each
                256KB t-block as soon as its sum lands, spread across
                DMA-capable queues so the drains overlap."""
                nc.vector.reciprocal(recips[:, i : i + 1], sums[:, i : i + 1])
                out16 = outpool.tile([128, _L], dt.bfloat16, tag="o16s")
                nc.vector.tensor_scalar_mul(out16[:], exp_sb[:], recips[:, i : i + 1])
                dst = out_r[:, i : i + 1, bb : bb + 1, :].squeeze(2).squeeze(1)
                eng = {4: nc.sync, 5: nc.gpsimd, 6: nc.sync, 7: nc.scalar}[i]
                eng.dma_start(dst, out16[:])

            # --- software-pipelined emission ---------------------------------
            enc = [None] * _BPC
            et = [None] * _BPC
            gt = [None] * _BPC
            eb = [None] * _BPC
            enc[0] = enc16_b0
            enc[1] = prep_load(1)  # first op on the SWDGE queue
            et[0] = prep_tr(0, enc[0], eng=nc.scalar)
            gt[0], ps_gs0 = prep_g_mm(0, et[0])
            prep_g_cast(gt[0], ps_gs0, 0)
            prep_g_cast(gt[0], ps_gs0, 1)
            eb[0] = prep_eb(0, et[0])

            pending = []
            for bb in range(_BPC):
                sums = sumpool.tile([128, 8], dt.float32, tag="sums")
                recips = sumpool.tile([128, 8], dt.float32, tag="recips")
                exps = [None] * 8
                last = bb == _BPC - 1
                for i in range(8):
                    exps[i] = chunk(bb, i, et[bb], gt[bb], eb[bb], sums)
                    if pending:
                        # deferred pair finish: emit AFTER the next chunk's
                        # diagonal write so the in-order DVE queue never
                        # makes ScalarE wait on normalize work
                        pending.pop()()
                    if last and i >= 4:
                        finish_chunk(bb, i, exps[i], sums, recips)
                    elif i % 2 == 1:
                        pending.append(
                            lambda bb=bb, q=i // 2, e=exps, s=sums, r=recips:
                                finish_pair(bb, q, e, s, r)
                        )
                    if bb + 1 < _BPC:
                        if i == 3:
                            et[bb + 1] = prep_tr(bb + 1, enc[bb + 1])
                        elif i == 4:
                            gt[bb + 1], ps_gs = prep_g_mm(bb + 1, et[bb + 1])
                            prep_g_cast(gt[bb + 1], ps_gs, 0)
                        elif i == 5:
                            prep_g_cast(gt[bb + 1], ps_gs, 1)
                        elif i == 6:
                            eb[bb + 1] = prep_eb(bb + 1, et[bb + 1])
                    if bb + 2 < _BPC and i == 2:
                        enc[bb + 2] = prep_load(bb + 2)

    nc.compile()
    _compiled_nc = nc
    return nc


def _numpy_fallback(enc, W, b, tl):
    energy = np.einsum("bld,ed->ble", enc, W) + b
    scores = np.einsum("bjd,btd->tbj", energy, enc[:, :tl, :])
    t_idx = np.arange(tl)
    scores[t_idx, :, t_idx] = 0.0
    m = scores.max(axis=-1, keepdims=True)
    e = np.exp(scores - m)
    return (e / e.sum(axis=-1, keepdims=True)).astype(np.float32)


def _run(encoder_outputs, W, b, target_length=1024, **run_kwargs):
    enc = np.ascontiguousarray(np.asarray(encoder_outputs, dtype=np.float32))
    Wn = np.ascontiguousarray(np.asarray(W, dtype=np.float32))
    bn = np.ascontiguousarray(np.asarray(b, dtype=np.float32))
    tl = int(target_length)
    if enc.shape != (_B, _L, _D) or tl != _T:
        return _numpy_fallback(enc, Wn, bn, tl), None

    from concourse.bass_utils import run_bass_kernel_spmd

    nc = _build()
    id16 = np.eye(128, dtype=np.float16)
    id8 = np.eye(128, dtype=np.int8)
    wt16 = Wn.T.astype(np.float16)  # [d, e]
    w4x2 = np.block([[wt16, wt16], [wt16, wt16]])  # [128, 128]
    nb16 = (-bn).astype(np.float16)
    nb22 = np.zeros((128, 2), dtype=np.float16)
    nb22[0:64, 0] = nb16
    nb22[64:128, 1] = nb16
    in_maps = [
        {
            "enc": enc[i * _BPC : (i + 1) * _BPC],
            "w4x2": w4x2,
            "nb22": nb22,
            "ident16": id16,
            "ident8": id8,
        }
        for i in range(_N_CORES)
    ]
    res = run_bass_kernel_spmd(nc, in_maps, list(range(_N_CORES)), **run_kwargs)
    out = np.concatenate(
        [np.asarray(res.results[i]["out"]) for i in range(_N_CORES)], axis=1
    ).astype(np.float32)
    return out, res


def kernel(encoder_outputs, W, b, target_length=1024):
    out, _ = _run(encoder_outputs, W, b, target_length)
    return out


def kernel_profiled(encoder_outputs, W, b, target_length=1024):
    """Run with NTFF tracing; returns (output, BassKernelResults)."""
    return _run(encoder_outputs, W, b, target_length, trace=True)


# revision 19
# speedup vs baseline: 1.0216x; 1.0154x over previous
"""Trainium2 Bass kernel for nn_Attn_45423574123081 (sparse_attention).

Computes, for inputs enc [B=32, L=1024, D=64], W [64, 64], b [64]:
    energy = enc @ W.T + b                       # [B, L, D]
    scores[t, b, j] = energy[b, j] . enc[b, t]   # [T=1024, B, L]
    scores[t, :, t] = 0
    out = softmax(scores, axis=-1)

Sharding: data-parallel over batch; 4 batches per core on 8 NeuronCores.

v2 redesign (from 70.1us baseline trace analysis):
 * Interleaved t-layout (t = 8p + n): the enc SWDGE cast-load becomes 128
   contiguous 2KB descriptors instead of 1024x256B (the old batch-0 load
   took ~10.5us and gated the first chunk at 17.3us).
 * Paired PE transposes [128,128]: even chunks' E^T on partitions 0-63,
   odd on 64-127.  K=64 matmuls then alternate tile_position row groups
   (0,0)/(64,0) between consecutive chunks, so neighbouring score
   matmuls run CONCURRENTLY in the PE array (~2x PE throughput; the
   old kernel's PE was 78% busy and stalled ScalarE at batch seams).
 * G^T built by one pair of concurrent matmuls with stationary
   [W^T | W^T] (output duplicated to both partition halves), written to
   two PSUM banks and interleave-merged f32->f16 by two strided DVE
   copies so gt columns are in true t'-order (output DMA stays 2KB-
   contiguous per partition).
 * eb = -(E.b) per chunk-pair via one K=128 matmul against a
   block-diagonal [[-b,0],[0,-b]] rhs.
 * Diagonal (softmax shift-invariance absorbs the +E.b bias): DVE
   predicated write of -c_t onto the stride-8 diagonal view ps[:, i::8].
 * Softmax unchanged: ScalarE exp f32 PSUM -> bf16 SBUF with fused
   accumulator row sums ('A') or DVE tensor_tensor_reduce ('T', to
   offload the ~290ns READ_ACCUMULATOR when ScalarE is the pacer),
   DVE reciprocal + normalize, 512KB bf16 DMA per pair of t-blocks.
 * Output rows t = 8p + i are stride-8 in DRAM but stay 2KB-contiguous
   per partition (partition stride 64KB).
"""

import numpy as np

_B, _L, _D, _T = 32, 1024, 64, 1024
_N_CORES = 8
_BPC = _B // _N_CORES  # batches per core

# Per-chunk row-sum mode: 'A' = ScalarE accumulator (+~290ns READ_ACC on
# ScalarE), 'T' = DVE tensor_tensor_reduce (~0.6us on DVE).
_MODES = ["A", "A", "A", "A", "A", "A", "A", "A"]

_compiled_nc = None


def _build():
    global _compiled_nc
    if _compiled_nc is not None:
        return _compiled_nc

    import concourse.bacc as bacc
    import concourse.mybir as mybir
    from concourse import tile

    dt = mybir.dt
    AF = mybir.ActivationFunctionType
    ALU = mybir.AluOpType

    nc = bacc.Bacc(
        "TRN2",
        target_bir_lowering=False,
        debug=False,
        enable_asserts=False,
        num_devices=_N_CORES,
    )
    enc_d = nc.dram_tensor("enc", [_BPC, _L, _D], dt.float32, kind="ExternalInput")
    # host-precomputed stationaries: [[W^T, W^T], [W^T, W^T]] and
    # [[-b, 0], [0, -b]] (removes the whole on-device W/b prep chain
    # from the startup critical path)
    w4_d = nc.dram_tensor("w4x2", [128, 128], dt.float16, kind="ExternalInput")
    nb_d = nc.dram_tensor("nb22", [128, 2], dt.float16, kind="ExternalInput")
    id16_d = nc.dram_tensor("ident16", [128, 128], dt.float16, kind="ExternalInput")
    id8_d = nc.dram_tensor("ident8", [128, 128], dt.int8, kind="ExternalInput")
    out_d = nc.dram_tensor("out", [_T, _BPC, _L], dt.bfloat16, kind="ExternalOutput")

    with tile.TileContext(nc) as tc:
        with (
            tc.tile_pool(name="const", bufs=1) as cpool,
            tc.tile_pool(name="encp", bufs=2) as encpool,
            tc.tile_pool(name="etp", bufs=2) as etpool,
            tc.tile_pool(name="gtp", bufs=2) as gtpool,
            tc.tile_pool(name="ebp", bufs=2) as ebpool,
            tc.tile_pool(name="expp", bufs=7) as exppool,
            tc.tile_pool(name="outp", bufs=3) as outpool,
            tc.tile_pool(name="sump", bufs=2) as sumpool,
            tc.tile_pool(name="scrp", bufs=2) as scrpool,
            tc.tile_pool(name="ps_s", bufs=3, space="PSUM") as ps_s_pool,
            tc.tile_pool(name="ps_m", bufs=2, space="PSUM") as ps_m_pool,
        ):
            # Dummy exp at t=0 hoists the ~2.7us ACT_TABLE_LOAD off the
            # first chunk's critical path.
            warm = cpool.tile([1, 2], dt.float32)
            nc.vector.memset(warm[:], 0.0)
            nc.scalar.activation(warm[:, 0:1], warm[:, 1:2], AF.Exp)

            # enc batch 0 goes f32 over the fast HWDGE sync queue (2KB
            # contiguous per partition) and is cast f32->f16 on the
            # still-idle DVE; batches 1-3 use the SWDGE casting loads
            # off the critical path.  Interleaved layout: partition p
            # holds rows t = 8p+0..8p+7.
            # enc32 first on the sync ring (single descriptor gen); the
            # masks go through the scalar (ACT) HWDGE ring in parallel
            # so the six ~650ns descriptor gens don't serialize.
            enc32_b0 = cpool.tile([128, 8 * _D], dt.float32)
            nc.sync.dma_start(
                enc32_b0[:].rearrange("p (n d) -> p n d", n=8),
                enc_d[0].rearrange("(p n) d -> p n d", p=128),
            )
            w4x2 = cpool.tile([128, 128], dt.float16)
            nc.sync.dma_start(w4x2[:], w4_d[:])
            nb22 = cpool.tile([128, 2], dt.float16)
            nc.sync.dma_start(nb22[:], nb_d[:])
            ident16 = cpool.tile([128, 128], dt.float16)
            nc.scalar.dma_start(ident16[:], id16_d[:])
            ident_i8 = cpool.tile([128, 128], dt.int8)
            nc.scalar.dma_start(ident_i8[:], id8_d[:])

            enc16_b0 = encpool.tile([128, 8 * _D], dt.float16, tag="enc16")
            for h in range(2):
                nc.vector.tensor_copy(
                    enc16_b0[:, h * 4 * _D : (h + 1) * 4 * _D],
                    enc32_b0[:, h * 4 * _D : (h + 1) * 4 * _D],
                )

            def prep_load(bb):
                """enc f32 DRAM -> f16 SBUF (cast on SWDGE), interleaved."""
                enc16 = encpool.tile([128, 8 * _D], dt.float16, tag="enc16")
                nc.gpsimd.dma_start(
                    enc16[:].rearrange("p (n d) -> p n d", n=8),
                    enc_d[bb].rearrange("(p n) d -> p n d", p=128),
                )
                return enc16

            def prep_tr(bb, enc16, eng=None):
                """4 paired transposes: et2 [128, 512] f16, pair q block
                at cols q*128..: rows 0-63 = E^T for chunk 2q (t=8p+2q),
                rows 64-127 = chunk 2q+1.  eng picks the PSUM->SBUF copy
                engine (ScalarE for batch 0, when it is still idle)."""
                ps_et = ps_m_pool.tile([128, 512], dt.float16, tag="ps_m")
                for q in range(4):
                    nc.tensor.transpose(
                        ps_et[:, q * 128 : (q + 1) * 128],
                        enc16[:, q * 128 : (q + 1) * 128],
                        ident16[:],
                    )
                et2 = etpool.tile([128, 512], dt.float16, tag="et2")
                if eng is None:
                    nc.vector.tensor_copy(et2[:], ps_et[:])
                else:
                    eng.copy(et2[:], ps_et[:])
                return et2

            def et_sl(et2, i):
                """lhsT slice [64, 128] for chunk i (row half i%2)."""
                q, h = i // 2, i % 2
                return et2[64 * h : 64 * h + 64, q * 128 : (q + 1) * 128]

            def prep_g_mm(bb, et2):
                """G MMs: two concurrent row-group MMs (even / odd
                chunks) into two PSUM banks, stacked (q,p) col order."""
                gt2 = gtpool.tile([128, _L], dt.float16, tag="gt2")
                ps_gs = []
                for h in range(2):
                    ps_g = ps_m_pool.tile([128, 512], dt.float32, tag="ps_m")
                    nc.tensor.matmul(
                        ps_g[:],
                        w4x2[64 * h : 64 * h + 64, :],
                        et2[64 * h : 64 * h + 64, :],
                        start=True,
                        stop=True,
                    )
                    ps_gs.append(ps_g)
                return gt2, ps_gs

            def prep_g_cast(gt2, ps_gs, h, eng=None):
                """Interleave-merge one parity into gt (t' = 8p+2q+h).
                Emitted one half per chunk slot so the 661ns casts never
                stack up between diag writes in the in-order DVE queue.
                src re-walked (q,p)->(p,q) to match the ascending-t'
                destination view."""
                gview = gt2[:].rearrange("p (pp h) -> p h pp", h=2)
                dst = gview[:, h : h + 1].squeeze(1)
                src = ps_gs[h][:].rearrange("p (q pp) -> p pp q", q=4)
                if eng is None:
                    nc.vector.tensor_copy(dst, src)
                else:
                    eng.copy(dst, src)

            def prep_eb(bb, et2):
                """ebn [128, 8] f32: col i = -c_t for chunk i rows
                (t = 8p + i).  One K=128 MM per chunk pair."""
                ps_eb = ps_m_pool.tile([128, 8], dt.float32, tag="ps_m")
                for q in range(4):
                    nc.tensor.matmul(
                        ps_eb[:, 2 * q : 2 * q + 2],
                        et2[:, q * 128 : (q + 1) * 128],
                        nb22[:],
                        start=True,
                        stop=True,
                    )
                ebn = ebpool.tile([128, 8], dt.float32, tag="ebn")
                nc.vector.tensor_copy(ebn[:], ps_eb[:])
                return ebn

            def chunk(bb, i, et2, gt2, ebn, sums):
                """One t-block: 2 score MMs (N=512 halves; consecutive
                chunks alternate PE row groups so they overlap), diag
                write on the stride-8 view, exp, row sum."""
                mode = _MODES[i]
                h = i % 2
                ps = ps_s_pool.tile([128, _L], dt.float32, tag="ps_s")
                for s in range(2):
                    sl = slice(s * 512, (s + 1) * 512)
                    nc.tensor.matmul(
                        ps[:, sl],
                        et_sl(et2, i),
                        gt2[64 * h : 64 * h + 64, sl],
                        start=True,
                        stop=True,
                    )
                # diagonal of chunk i sits at (p, 8p + i): stride-8 view
                diag_view = ps[:].rearrange("p (pp e) -> p e pp", e=8)[
                    :, i : i + 1
                ].squeeze(1)
                nc.vector.copy_predicated(
                    diag_view,
                    ident_i8[:],
                    ebn[:, i : i + 1].to_broadcast([128, 128]),
                )
                exp_sb = exppool.tile([128, _L], dt.bfloat16, tag="exp")
                scol = sums[:, i : i + 1]
                if mode == "A":
                    nc.scalar.activation(exp_sb[:], ps[:], AF.Exp, accum_out=scol)
                else:
                    nc.scalar.activation(exp_sb[:], ps[:], AF.Exp)
                    scr = scrpool.tile([128, 512], dt.bfloat16, tag="scr")
                    nc.vector.tensor_tensor_reduce(
                        scr[:],
                        exp_sb[:, 0:512],
                        exp_sb[:, 512:1024],
                        1.0,
                        0.0,
                        ALU.add,
                        ALU.add,
                        accum_out=scol,
                    )
                return exp_sb

            out_r = out_d.rearrange("(p e) b j -> p e b j", e=8)

            def finish_pair(bb, q, exps, sums, recips):
                """Reciprocal for chunks 2q/2q+1, normalize, DMA out."""
                pr = slice(2 * q, 2 * q + 2)
                nc.vector.reciprocal(recips[:, pr], sums[:, pr])
                out16 = outpool.tile([128, 2 * _L], dt.bfloat16, tag="o16")
                for h in range(2):
                    i = 2 * q + h
                    nc.vector.tensor_scalar_mul(
                        out16[:, h * _L : (h + 1) * _L],
                        exps[i][:],
                        recips[:, i : i + 1],
                    )
                dst = out_r[:, 2 * q : 2 * q + 2, bb : bb + 1, :].squeeze(2)
                nc.sync.dma_start(dst, out16[:].rearrange("p (e j) -> p e j", e=2))

            def finish_chunk(bb, i, exp_sb, sums, recips):
                """Tail-latency variant for the last batch: ship each
                256KB t-block as soon as its sum lands, spread across
                DMA-capable queues so the drains overlap."""
                nc.vector.reciprocal(recips[:, i : i + 1], sums[:, i : i + 1])
                out16 = outpool.tile([128, _L], dt.bfloat16, tag="o16s")
                nc.vector.tensor_scalar_mul(out16[:], exp_sb[:], recips[:, i : i + 1])
                dst = out_r[:, i : i + 1, bb : bb + 1, :].squeeze(2).squeeze(1)
                eng = {4: nc.sync, 5: nc.gpsimd, 6: nc.sync, 7: nc.scalar}[i]
                eng.dma_start(dst, out16[:])

            # --- software-pipelined emission ---------------------------------
            enc = [None] * _BPC
            et = [None] * _BPC
            gt = [None] * _BPC
            eb = [None] * _BPC
            enc[0] = enc16_b0
            enc[1] = prep_load(1)  # first op on the SWDGE queue
            et[0] = prep_tr(0, enc[0], eng=nc.scalar)
            gt[0], ps_gs0 = prep_g_mm(0, et[0])
            prep_g_cast(gt[0], ps_gs0, 0, eng=nc.scalar)
            prep_g_cast(gt[0], ps_gs0, 1)
            eb[0] = prep_eb(0, et[0])

            pending = []
            for bb in range(_BPC):
                sums = sumpool.tile([128, 8], dt.float32, tag="sums")
                recips = sumpool.tile([128, 8], dt.float32, tag="recips")
                exps = [None] * 8
                last = bb == _BPC - 1
                for i in range(8):
                    exps[i] = chunk(bb, i, et[bb], gt[bb], eb[bb], sums)
                    # deferred pair finish: emit ~2 chunks after the pair
                    # completes so the in-order DVE queue never makes
                    # ScalarE wait on bunched normalize work (1-deep on
                    # the last batch to drain promptly)
                    if len(pending) >= (1 if last else 2):
                        pending.pop(0)()
                    if last and i >= 4:
                        finish_chunk(bb, i, exps[i], sums, recips)
                    elif i % 2 == 1:
                        pending.append(
                            lambda bb=bb, q=i // 2, e=exps, s=sums, r=recips:
                                finish_pair(bb, q, e, s, r)
                        )
                    if bb + 1 < _BPC:
                        if i == 3:
                            et[bb + 1] = prep_tr(bb + 1, enc[bb + 1])
                        elif i == 4:
                            gt[bb + 1], ps_gs = prep_g_mm(bb + 1, et[bb + 1])
                            prep_g_cast(gt[bb + 1], ps_gs, 0)
                        elif i == 5:
                            prep_g_cast(gt[bb + 1], ps_gs, 1)
                        elif i == 6:
                            eb[bb + 1] = prep_eb(bb + 1, et[bb + 1])
                    if bb + 2 < _BPC and i == 2:
                        enc[bb + 2] = prep_load(bb + 2)

    nc.compile()
    _compiled_nc = nc
    return nc


def _numpy_fallback(enc, W, b, tl):
    energy = np.einsum("bld,ed->ble", enc, W) + b
    scores = np.einsum("bjd,btd->tbj", energy, enc[:, :tl, :])
    t_idx = np.arange(tl)
    scores[t_idx, :, t_idx] = 0.0
    m = scores.max(axis=-1, keepdims=True)
    e = np.exp(scores - m)
    return (e / e.sum(axis=-1, keepdims=True)).astype(np.float32)


def _run(encoder_outputs, W, b, target_length=1024, **run_kwargs):
    enc = np.ascontiguousarray(np.asarray(encoder_outputs, dtype=np.float32))
    Wn = np.ascontiguousarray(np.asarray(W, dtype=np.float32))
    bn = np.ascontiguousarray(np.asarray(b, dtype=np.float32))
    tl = int(target_length)
    if enc.shape != (_B, _L, _D) or tl != _T:
        return _numpy_fallback(enc, Wn, bn, tl), None

    from concourse.bass_utils import run_bass_kernel_spmd

    nc = _build()
    id16 = np.eye(128, dtype=np.float16)
    id8 = np.eye(128, dtype=np.int8)
    wt16 = Wn.T.astype(np.float16)  # [d, e]
    w4x2 = np.block([[wt16, wt16], [wt16, wt16]])  # [128, 128]
    nb16 = (-bn).astype(np.float16)
    nb22 = np.zeros((128, 2), dtype=np.float16)
    nb22[0:64, 0] = nb16
    nb22[64:128, 1] = nb16
    in_maps = [
        {
            "enc": enc[i * _BPC : (i + 1) * _BPC],
            "w4x2": w4x2,
            "nb22": nb22,
            "ident16": id16,
            "ident8": id8,
        }
        for i in range(_N_CORES)
    ]
    res = run_bass_kernel_spmd(nc, in_maps, list(range(_N_CORES)), **run_kwargs)
    out = np.concatenate(
        [np.asarray(res.results[i]["out"]) for i in range(_N_CORES)], axis=1
    ).astype(np.float32)
    return out, res


def kernel(encoder_outputs, W, b, target_length=1024):
    out, _ = _run(encoder_outputs, W, b, target_length)
    return out


def kernel_profiled(encoder_outputs, W, b, target_length=1024):
    """Run with NTFF tracing; returns (output, BassKernelResults)."""
    return _run(encoder_outputs, W, b, target_length, trace=True)
